# revision 1
# baseline (speedup 1.0000x reference)
"""Trainium2 Bass kernel for the PINN-style loss problem.

Math: a 6-layer tanh MLP u(x,t) (2->50x5->1) is evaluated with forward-mode
jets (u, u_x, u_t, u_xxx) at N=10000 points. The per-param loss
  loss_p = mean_n (u_t + a_p*u*u_x + b_p*u_xxx + c_p*u_x)^2
collapses to a quadratic form in the 4x4 Gram matrix of
g_n = [u*u_x, u_xxx, u_x, u_t]:  loss_p = (s0 + 2 p.s1 + p^T S p)/N.

Sharding: x is split into 8 slices of 1250 points (one per NeuronCore);
each core builds its partial Gram, an AllReduce sums them, then each core
evaluates the quadratic form for its 625-row slice of para.

Device layout: points are packed 2-per-partition-block (block-diagonal
weights, K=100), free dim 640 per block (block0: 640 real points,
block1: 610 real + 30 zero-padded, masked out before the Gram matmul).
"""

import os
import sys
import numpy as np

for _p in ("/opt/trn_rl_repo",):
    if os.path.isdir(_p) and _p not in sys.path:
        sys.path.append(_p)

import concourse.bass as bass
import concourse.bacc as bacc
import concourse.mybir as mybir
import concourse.tile as tile
from concourse import bass_utils

F32 = mybir.dt.float32
F32R = mybir.dt.float32r
AF = mybir.ActivationFunctionType
ALU = mybir.AluOpType

NCORES = 8
NPTS = 10000
NPC = NPTS // NCORES       # 1250 points per core
PPC = 5000 // NCORES       # 625 para rows per core
FD = 640                   # free dim per block (block0 full, block1 padded)
B1 = NPC - FD              # 610 real points in block1
HB = 100                   # 2 blocks x 50 hidden units
CHUNKS = ((0, 512), (512, 128))      # matmul free-dim chunks (psum bank limit)
PCH = ((0, 512), (512, PPC - 512))   # para free-dim chunks

SDT = F32R                 # tower stream/weight dtype (f32r: 1 cyc/row matmul)
WARM_CC = True             # early dummy collective to warm the CC path


def _mm(nc, out, lhsT, rhs, start=True, stop=True):
    nc.tensor.matmul(out, lhsT, rhs, start=start, stop=stop)


def _mm_chunks(nc, out_tile, lhsT, rhs_tile, chunks=CHUNKS):
    for off, w in chunks:
        _mm(nc, out_tile[:, off:off + w], lhsT, rhs_tile[:, off:off + w])


def build_program(stage="full"):
    nc = bacc.Bacc("TRN2", target_bir_lowering=False, debug=False)

    h0_d = nc.dram_tensor("h0", [4, FD], SDT, kind="ExternalInput")
    paraT_d = nc.dram_tensor("paraT", [3, PPC], F32, kind="ExternalInput")
    w1t_d = nc.dram_tensor("w1t", [4, HB], SDT, kind="ExternalInput")
    wb_d = nc.dram_tensor("wb", [HB, 400], SDT, kind="ExternalInput")
    w6p_d = nc.dram_tensor("w6p", [HB, 2], SDT, kind="ExternalInput")
    vecs_d = nc.dram_tensor("vecs", [HB, 10], F32, kind="ExternalInput")
    b6bc_d = nc.dram_tensor("b6bc", [128, 2], F32, kind="ExternalInput")
    if stage == "full":
        loss_d = nc.dram_tensor("loss", [1, PPC], F32, kind="ExternalOutput")
    elif stage == "tower":
        loss_d = nc.dram_tensor("dbg", [HB, FD], F32, kind="ExternalOutput")
    elif stage == "para2":
        loss_d = nc.dram_tensor("loss", [1, PPC], F32, kind="ExternalOutput")
    else:  # l6 / cc / para1
        loss_d = nc.dram_tensor("dbg", [5, 4], F32, kind="ExternalOutput")

    with tile.TileContext(nc) as tc:
        _body(tc, nc, h0_d, paraT_d, w1t_d, wb_d, w6p_d, vecs_d, b6bc_d, loss_d,
              stage=stage)
    nc.compile()
    return nc


def _body(tc, nc, h0_d, paraT_d, w1t_d, wb_d, w6p_d, vecs_d, b6bc_d, loss_d,
          stage="full"):
    import contextlib

    ctx = contextlib.ExitStack()
    with ctx:
        cpool = ctx.enter_context(tc.tile_pool(name="const", bufs=1))
        spool = ctx.enter_context(tc.tile_pool(name="streams", bufs=2))
        tpool = ctx.enter_context(tc.tile_pool(name="trans", bufs=2))
        dpool = ctx.enter_context(tc.tile_pool(name="dram", bufs=1, space="DRAM"))

        # ---- load constants ----
        h0 = cpool.tile([4, FD], SDT, tag="h0")
        paraT = cpool.tile([3, PPC], F32, tag="paraT")
        w1t = cpool.tile([4, HB], SDT, tag="w1t")
        wb = cpool.tile([HB, 400], SDT, tag="wb")
        w6p = cpool.tile([HB, 2], SDT, tag="w6p")
        vecs = cpool.tile([HB, 10], F32, tag="vecs")
        b6bc = cpool.tile([128, 2], F32, tag="b6bc")
        ones3 = cpool.tile([3, 1], F32, tag="ones3")
        for t, d in ((h0, h0_d), (paraT, paraT_d), (w1t, w1t_d), (wb, wb_d),
                     (w6p, w6p_d), (vecs, vecs_d), (b6bc, b6bc_d)):
            nc.sync.dma_start(t[:], d[:])
        nc.vector.memset(ones3[:], 1.0)

        if WARM_CC:
            win = dpool.tile([1, 1], F32, tag="win")
            wout = dpool.tile([1, 1], F32, tag="wout")
            nc.gpsimd.dma_start(win[:], ones3[0:1, 0:1])
            nc.gpsimd.collective_compute(
                "AllReduce", ALU.add,
                replica_groups=[list(range(NCORES))],
                ins=[win.opt()], outs=[wout.opt()],
            )

        cx = vecs[:, 0:1]
        ct = vecs[:, 1:2]
        cx2 = vecs[:, 2:3]
        cx3 = vecs[:, 3:4]

        def bb(layer):  # bias vector for layer 1..5
            return vecs[:, 3 + layer:4 + layer]

        neg2 = vecs[:, 9:10]

        v = nc.vector
        s = nc.scalar
        g = nc.gpsimd

        a5 = ax5 = at5 = axxx5 = None

        with tc.tile_pool(name="ztw", bufs=3, space="PSUM") as zpool:
            # ---------- layer 1 ----------
            z = zpool.tile([HB, FD], F32, tag="ztw")
            _mm_chunks(nc, z, w1t[:], h0)
            a = spool.tile([HB, FD], SDT, tag="a")
            s.activation(a[:], z[:], AF.Tanh, bias=bb(1))
            asq = tpool.tile([HB, FD], F32, tag="asq")
            s.activation(asq[:], a[:], AF.Square)
            f1 = tpool.tile([HB, FD], F32, tag="f1")
            s.activation(f1[:], asq[:], AF.Identity, scale=-1.0, bias=1.0)
            h6 = tpool.tile([HB, FD], F32, tag="h6")
            s.activation(h6[:], asq[:], AF.Identity, scale=6.0, bias=neg2)
            ax = spool.tile([HB, FD], SDT, tag="ax")
            v.tensor_scalar(ax[:], f1[:], cx, None, ALU.mult)
            at = spool.tile([HB, FD], SDT, tag="at")
            v.tensor_scalar(at[:], f1[:], ct, None, ALU.mult)
            af1 = tpool.tile([HB, FD], F32, tag="p1")
            v.tensor_tensor(af1[:], a[:], f1[:], ALU.mult)
            axx = spool.tile([HB, FD], SDT, tag="axx")
            v.tensor_scalar(axx[:], af1[:], cx2, -2.0, ALU.mult, ALU.mult)
            f3 = tpool.tile([HB, FD], F32, tag="p2")
            g.tensor_tensor(f3[:], f1[:], h6[:], ALU.mult)
            axxx = spool.tile([HB, FD], SDT, tag="axxx")
            v.tensor_scalar(axxx[:], f3[:], cx3, None, ALU.mult)

            # ---------- layers 2..5 ----------
            for layer in range(2, 6):
                W = wb[:, 100 * (layer - 2):100 * (layer - 1)]
                last = layer == 5

                z = zpool.tile([HB, FD], F32, tag="ztw")
                _mm_chunks(nc, z, W, a)
                a_n = spool.tile([HB, FD], SDT, tag="a")
                s.activation(a_n[:], z[:], AF.Tanh, bias=bb(layer))

                zt = zpool.tile([HB, FD], F32, tag="ztw")
                _mm_chunks(nc, zt, W, at)
                asq = tpool.tile([HB, FD], F32, tag="asq")
                s.activation(asq[:], a_n[:], AF.Square)
                f1 = tpool.tile([HB, FD], F32, tag="f1")
                s.activation(f1[:], asq[:], AF.Identity, scale=-1.0, bias=1.0)
                at_n = spool.tile([HB, FD], SDT, tag="at")
                v.tensor_tensor(at_n[:], f1[:], zt[:], ALU.mult)

                zx = zpool.tile([HB, FD], F32, tag="ztw")
                _mm_chunks(nc, zx, W, ax)
                h6 = tpool.tile([HB, FD], F32, tag="h6")
                s.activation(h6[:], asq[:], AF.Identity, scale=6.0, bias=neg2)
                ax_n = spool.tile([HB, FD], SDT, tag="ax")
                v.tensor_tensor(ax_n[:], f1[:], zx[:], ALU.mult)
                w2 = tpool.tile([HB, FD], F32, tag="w2")
                s.activation(w2[:], zx[:], AF.Square)
                P = tpool.tile([HB, FD], F32, tag="p1")
                v.tensor_tensor(P[:], a_n[:], zx[:], ALU.mult)
                zx3 = tpool.tile([HB, FD], F32, tag="zx3")
                v.tensor_tensor(zx3[:], w2[:], zx[:], ALU.mult)

                zxx = zpool.tile([HB, FD], F32, tag="ztw")
                _mm_chunks(nc, zxx, W, axx)
                if not last:
                    gt = tpool.tile([HB, FD], F32, tag="g")
                    g.tensor_tensor(gt[:], a_n[:], w2[:], ALU.mult)
                    inner = tpool.tile([HB, FD], F32, tag="inner")
                    v.scalar_tensor_tensor(inner[:], gt[:], -2.0, zxx[:],
                                           ALU.mult, ALU.add)
                m = tpool.tile([HB, FD], F32, tag="p2")
                v.tensor_tensor(m[:], P[:], zxx[:], ALU.mult)
                if not last:
                    axx_n = spool.tile([HB, FD], SDT, tag="axx")
                    g.tensor_tensor(axx_n[:], f1[:], inner[:], ALU.mult)

                zxxx = zpool.tile([HB, FD], F32, tag="ztw")
                _mm_chunks(nc, zxxx, W, axxx)
                i3a = tpool.tile([HB, FD], F32, tag="i3a")
                v.scalar_tensor_tensor(i3a[:], m[:], -6.0, zxxx[:],
                                       ALU.mult, ALU.add)
                n_t = tpool.tile([HB, FD], F32, tag="n")
                g.tensor_tensor(n_t[:], h6[:], zx3[:], ALU.mult)
                i3 = tpool.tile([HB, FD], F32, tag="i3")
                g.tensor_tensor(i3[:], i3a[:], n_t[:], ALU.add)
                axxx_n = spool.tile([HB, FD], SDT, tag="axxx")
                v.tensor_tensor(axxx_n[:], f1[:], i3[:], ALU.mult)

                a, at, ax, axxx = a_n, at_n, ax_n, axxx_n
                if not last:
                    axx = axx_n

            a5, ax5, at5, axxx5 = a, ax, at, axxx

        if stage == "tower":
            nc.sync.dma_start(loss_d[:], axxx5[:].bitcast(F32))
            return

        # ---------- layer 6 + Gram ----------
        # chunk tiles: [128 points, 10] cols: s-major pairs (b0,b1) for
        # s=0 uux, 1 uxxx, 2 ux, 3 ut; cols 8:10 = u.
        with tc.tile_pool(name="l6c", bufs=2, space="PSUM") as l6p, \
             tc.tile_pool(name="psmall", bufs=1, space="PSUM") as pps:
            G = pps.tile([4, 4], F32, tag="gram")
            Gr4 = pps.tile([1, 4], F32, tag="gram_r")
            for c in range(5):
                lo = 128 * c
                ch = l6p.tile([128, 10], F32, tag="l6c")
                _mm(nc, ch[:, 8:10], a5[:, lo:lo + 128], w6p[:])
                _mm(nc, ch[:, 2:4], axxx5[:, lo:lo + 128], w6p[:])
                _mm(nc, ch[:, 4:6], ax5[:, lo:lo + 128], w6p[:])
                _mm(nc, ch[:, 6:8], at5[:, lo:lo + 128], w6p[:])
                chS = tpool.tile([128, 10], F32, tag="l6s")
                v.tensor_copy(chS[:, 2:10], ch[:, 2:10])
                # uux = (u + b6) * ux
                v.scalar_tensor_tensor(chS[:, 0:2], chS[:, 8:10], b6bc[:128, 0:1],
                                       chS[:, 4:6], ALU.add, ALU.mult)
                chv = chS[:, 0:8].rearrange("p (s b) -> p b s", b=2, s=4)
                if c == 4 and B1 < FD:
                    # zero the padded block1 points before the Gram matmul
                    v.tensor_scalar(chv[:, 1, :], chv[:, 1, :], b6bc[:128, 1:2],
                                    None, ALU.mult)
                for b in range(2):
                    st = c == 0 and b == 0
                    sp = c == 4 and b == 1
                    nc.tensor.matmul(G[:], chv[:, b, :], chv[:, b, :],
                                     start=st, stop=sp)
                    # last Gram row (incl s0 = sum ut^2) at partition 0
                    nc.tensor.matmul(Gr4[:], chv[:, b, 3:4], chv[:, b, :],
                                     start=st, stop=sp)

            gS = cpool.tile([4, 4], F32, tag="gS")
            v.tensor_copy(gS[:], G[:])
            gS4 = cpool.tile([1, 4], F32, tag="gS4")
            v.tensor_copy(gS4[:], Gr4[:])

            if stage == "l6":
                nc.sync.dma_start(loss_d[0:4, :], gS[:])
                nc.sync.dma_start(loss_d[4:5, :], gS4[:])
                return

            # ---------- AllReduce the Gram (packed [5,4] bounce) ----------
            gin = dpool.tile([5, 4], F32, tag="gin")
            gout = dpool.tile([5, 4], F32, tag="gout")
            nc.gpsimd.dma_start(gin[0:4, :], gS[:])
            nc.gpsimd.dma_start(gin[4:5, :], gS4[:])
            nc.gpsimd.collective_compute(
                "AllReduce",
                ALU.add,
                replica_groups=[list(range(NCORES))],
                ins=[gin.opt()],
                outs=[gout.opt()],
            )
            Gr = cpool.tile([4, 4], F32, tag="Gr")
            nc.gpsimd.dma_start(Gr[:], gout[0:4, :])
            GrR = cpool.tile([1, 4], F32, tag="GrR")
            nc.gpsimd.dma_start(GrR[:], gout[4:5, :])

            if stage == "cc":
                nc.sync.dma_start(loss_d[0:4, :], Gr[:])
                nc.sync.dma_start(loss_d[4:5, :], GrR[:])
                return

            # ---------- para quadratic form ----------
            # loss = (s0 + 2 p.s1 + p^T S p) / N
            # S = Gr[0:3,0:3], s1 = Gr[0:3,3], s0 = GrR[0,3]
            s1d = cpool.tile([3, 1], F32, tag="s1d")
            s.activation(s1d[:], Gr[0:3, 3:4], AF.Copy, scale=2.0)
            PS = pps.tile([3, PPC], F32, tag="PS")
            for off, w in PCH:
                _mm(nc, PS[:, off:off + w], Gr[0:3, 0:3], paraT[:, off:off + w])
            H3 = cpool.tile([3, PPC], F32, tag="H3")
            v.scalar_tensor_tensor(H3[:], PS[:], s1d[:], paraT[:],
                                   ALU.add, ALU.mult)
            if stage == "para1":
                nc.sync.dma_start(loss_d[0:3, :], H3[:, 0:4])
                nc.sync.dma_start(loss_d[3:4, :], GrR[:])
                return
            LP = pps.tile([1, PPC], F32, tag="LP")
            for off, w in PCH:
                _mm(nc, LP[:, off:off + w], ones3[:], H3[:, off:off + w])
            lossS = cpool.tile([1, PPC], F32, tag="lossS")
            # loss = (LP + s0) / N  -- s0 folded via DVE scalar-AP add
            v.tensor_scalar(lossS[:], LP[:], GrR[0:1, 3:4], 1.0 / NPTS,
                            ALU.add, ALU.mult)
            nc.sync.dma_start(loss_d[:], lossS[:])


def prep_inputs(x, para, W1, b1, W2, b2, W3, b3, W4, b4, W5, b5, W6, b6):
    """Full inputs -> list of per-core input dicts (host-side shard/layout)."""
    f = np.float32
    x = np.asarray(x, f)
    para = np.asarray(para, f)
    Ws = [np.asarray(W, f) for W in (W1, W2, W3, W4, W5, W6)]
    bs = [np.asarray(b, f) for b in (b1, b2, b3, b4, b5, b6)]

    w1t = np.zeros((4, HB), f)
    w1t[0:2, 0:50] = Ws[0].T
    w1t[2:4, 50:100] = Ws[0].T
    wb = np.zeros((HB, 400), f)
    for i in range(4):
        W = Ws[i + 1]
        wb[0:50, 100 * i:100 * i + 50] = W.T
        wb[50:100, 100 * i + 50:100 * i + 100] = W.T
    w6p = np.zeros((HB, 2), f)
    w6p[0:50, 0] = Ws[5][0]
    w6p[50:100, 1] = Ws[5][0]
    vecs = np.zeros((HB, 10), f)
    vecs[:, 9] = -2.0
    cx = Ws[0][:, 0]
    ct = Ws[0][:, 1]
    for half in (slice(0, 50), slice(50, 100)):
        vecs[half, 0] = cx
        vecs[half, 1] = ct
        vecs[half, 2] = cx * cx
        vecs[half, 3] = cx * cx * cx
        for l in range(5):
            vecs[half, 4 + l] = bs[l]
    b6bc = np.zeros((128, 2), f)
    b6bc[:, 0] = bs[5][0]
    b6bc[:, 1] = 1.0
    b6bc[B1 - 512:, 1] = 0.0

    maps = []
    for c in range(NCORES):
        sl = x[c * NPC:(c + 1) * NPC]
        h0 = np.zeros((4, FD), f)
        h0[0] = sl[0:FD, 0]
        h0[1] = sl[0:FD, 1]
        h0[2, 0:B1] = sl[FD:NPC, 0]
        h0[3, 0:B1] = sl[FD:NPC, 1]
        paraT = np.ascontiguousarray(para[c * PPC:(c + 1) * PPC].T)
        maps.append({
            "h0": h0, "paraT": paraT, "w1t": w1t, "wb": wb,
            "w6p": w6p, "vecs": vecs, "b6bc": b6bc,
        })
    return maps


_NC_CACHE = {}


def get_program():
    if "nc" not in _NC_CACHE:
        _NC_CACHE["nc"] = build_program()
    return _NC_CACHE["nc"]


def kernel(x, para, W1, b1, W2, b2, W3, b3, W4, b4, W5, b5, W6, b6):
    maps = prep_inputs(x, para, W1, b1, W2, b2, W3, b3, W4, b4, W5, b5, W6, b6)
    nc = get_program()
    res = bass_utils.run_bass_kernel_spmd(nc, maps, list(range(NCORES)))
    out = np.concatenate([res.results[c]["loss"].reshape(-1) for c in range(NCORES)])
    return out.astype(np.float32)

